# revision 1
# baseline (speedup 1.0000x reference)
"""Chamfer loss kernel for 8 Trainium2 NeuronCores.

Problem: ground_truth [4, 8192, 3], reconstruction [4, 8192, 3] (fp32).
  P[b,n,m] = ||x_n||^2 + ||y_m||^2 - 2 x_n.y_m
  loss = (mean(clamp(min_n P)) + mean(clamp(min_m P))) * 1000

Sharding: 8 independent (direction, batch) units -> 1 per core.
  cores 0..3: a = ground_truth[b],  b = reconstruction[b]   (loss_2: min over m)
  cores 4..7: a = reconstruction[b], b = ground_truth[b]    (loss_1: min over n)
Each core returns per-partition partial sums of clamp(min_b dist^2(a_i, b))
over its 8192 a-points; the host sums and combines.

Per-core kernel:
  - min_b(xx + yy - 2xy) = xx + min_b(yy - 2xy): xx is constant over the min
    axis, so the matmul only computes P' = yy - 2xy with K=4:
    lhsT rows [x0,x1,x2,1], rhs rows [-2y0,-2y1,-2y2,yy].
  - 64 a-tiles x 16 b-chunks of [128, 512] matmuls into PSUM, with 4 chunks
    packed concurrently into the PE array via tile_position row groups
    (K=4 uses 4 of 128 rows; operands replicated at partitions 0/32/64/96),
    recovering ~4x fp32 matmul throughput.
  - min-reduce: ScalarE copies one bank of each pair to SBUF; VectorE
    tensor_tensor_scan(op0=min, op1=min) consumes (psum, sbuf) pairs at
    ~2 elements/lane/cycle; the scan's last column is the running min and
    feeds the next scan's `initial` directly (no extract ops).
    (tensor_tensor_reduce would fuse this in one op but crashes TRN2
    silicon; two-PSUM-operand DVE ops are rejected by the compiler.)
  - finalize per a-tile: min + xx, clamp at 1e-10; final free-axis sum
    -> [128, 1] per-partition partial output, combined on host.
"""

import sys

if "/opt/trn_rl_repo" not in sys.path:
    sys.path.insert(0, "/opt/trn_rl_repo")

from contextlib import ExitStack

import numpy as np

N = 8192
D = 3
P = 128
NT = N // P  # 64 a-tiles
CH = 512
NCH = N // CH  # 16 b-chunks
PAIRS = NCH // 2

TRACE = False  # set True from test harness to capture an NTFF profile
LAST_RESULTS = None  # BassKernelResults of the most recent run (when traced)

_CACHE = {}


def _build_nc(
    mm_dtype_name="float32", nt_main=NT, skip_dma=False, variant="scan_pack", reps=1
):
    import concourse.bacc as bacc
    import concourse.tile as tile
    from concourse import mybir
    from concourse.masks import make_identity

    f32 = mybir.dt.float32
    mm_dt = getattr(mybir.dt, mm_dtype_name)

    nc = bacc.Bacc("TRN2", target_bir_lowering=False, debug=False)

    a_dram = nc.dram_tensor("a_pts", [N, D], f32, kind="ExternalInput")
    b_dram = nc.dram_tensor("b_pts", [N, D], f32, kind="ExternalInput")
    out_dram = nc.dram_tensor("partial", [P, 1], f32, kind="ExternalOutput")

    pack = variant == "scan_pack"
    with tile.TileContext(nc) as tc, ExitStack() as ctx:
        consts = ctx.enter_context(tc.tile_pool(name="consts", bufs=1))
        sb = ctx.enter_context(tc.tile_pool(name="sb", bufs=1))
        small = ctx.enter_context(tc.tile_pool(name="small", bufs=2))
        scratch = ctx.enter_context(tc.tile_pool(name="scratch", bufs=3))
        sq3p = ctx.enter_context(tc.tile_pool(name="sq3p", bufs=2))
        prep_ctx = ExitStack()
        prep_ps = prep_ctx.enter_context(tc.tile_pool(name="prep_ps", bufs=2, space="PSUM"))
        yy_ps = prep_ctx.enter_context(tc.tile_pool(name="yy_ps", bufs=1, space="PSUM"))
        main_ps = None  # opened after prep pools close when pack=True

        ident = consts.tile([P, P], f32)
        make_identity(nc, ident)
        # ones_mask [3, 4]: col 3 = 1s, cols 0:2 = 0 -> ones_mask.T @ sq3 puts
        # yy into row 3 and 0 into rows 0:2
        ones_mask = consts.tile([3, 4], f32)
        nc.vector.memset(ones_mask, 0.0)
        nc.vector.memset(ones_mask[:, 3:4], 1.0)
        # per-partition scale [-2, -2, -2, 1] for the RHS write
        sv = consts.tile([4, 1], f32)
        nc.vector.memset(sv, -2.0)
        nc.gpsimd.affine_select(
            out=sv,
            in_=sv,
            compare_op=mybir.AluOpType.not_equal,
            fill=1.0,
            base=-3,
            pattern=[[0, 1]],
            channel_multiplier=1,
        )

        # natural-layout staging: [128 points-in-tile, 64 tiles, 4] where the
        # 4th pseudo-coordinate is 1.0 (a-side) / 0.0 (b-side)
        astage = sb.tile([P, NT, 4], f32)
        nc.vector.memset(astage[:, :, 3:4], 1.0)
        bstage = sb.tile([P, NT, 4], f32)
        nc.vector.memset(bstage[:, :, 3:4], 0.0)
        if skip_dma:
            nc.vector.memset(astage[:, :, 0:D], 0.5)
            nc.vector.memset(bstage[:, :, 0:D], 0.25)
        else:
            nc.sync.dma_start(
                out=astage[:, :, 0:D],
                in_=a_dram.ap().rearrange("(t p) d -> p t d", p=P),
            )
            nc.sync.dma_start(
                out=bstage[:, :, 0:D],
                in_=b_dram.ap().rearrange("(t p) d -> p t d", p=P),
            )

        # xx per a-point, natural layout [128, 64]
        sqa = sb.tile([P, NT, D], f32)
        nc.vector.tensor_mul(sqa, astage[:, :, 0:D], astage[:, :, 0:D])
        xx = sb.tile([P, NT], f32)
        nc.vector.tensor_reduce(
            out=xx, in_=sqa, axis=mybir.AxisListType.X, op=mybir.AluOpType.add
        )

        # K-major operand buffers (full 128 partitions when packing: rows
        # replicated at partition bases 0/32/64/96)
        LHS = sb.tile([P if pack else 4, N], mm_dt)  # rows: x0, x1, x2, 1
        RHS = sb.tile([P if pack else 4, N], mm_dt)  # rows: -2y0, -2y1, -2y2, yy

        # build LHS (a-coords + ones row) and RHS (b-side) per 512-col group
        for g in range(NCH):
            # a-side: transpose 4 [128,4] tiles -> psum [4, 512], copy to LHS
            tpa = prep_ps.tile([4, CH], f32, tag="tp")
            for c in range(4):
                t = 4 * g + c
                nc.tensor.transpose(tpa[:, c * P : (c + 1) * P], astage[:, t, :], ident)
            nc.scalar.copy(LHS[0:4, g * CH : (g + 1) * CH], tpa)

            # b-side: transpose -> [y0,y1,y2,0]; square rows 0:3; accumulate
            # yy into row 3 via ones_mask matmul; write RHS with scale vec
            tpb = prep_ps.tile([4, CH], f32, tag="tp")
            for c in range(4):
                t = 4 * g + c
                nc.tensor.transpose(tpb[:, c * P : (c + 1) * P], bstage[:, t, :], ident)
            sq3 = sq3p.tile([3, CH], f32)
            nc.scalar.square(sq3, tpb[0:3, :])
            ypb = yy_ps.tile([4, CH], f32)
            nc.tensor.matmul(ypb, ones_mask, sq3, start=True, stop=True)
            ypb_sb = sq3p.tile([4, CH], f32, tag="ypb_sb")
            nc.scalar.copy(ypb_sb, ypb)
            # RHS = tpb * [-2,-2,-2,1] + [0,0,0,yy]  (<=1 PSUM operand per op)
            nc.vector.scalar_tensor_tensor(
                out=RHS[0:4, g * CH : (g + 1) * CH],
                in0=tpb,
                scalar=sv,
                in1=ypb_sb,
                op0=mybir.AluOpType.mult,
                op1=mybir.AluOpType.add,
            )

        if pack:
            # replicate operand rows to partition bases 32/64/96 for
            # tile_position row-group packing (4 concurrent K=4 matmuls)
            for r in (32, 64, 96):
                nc.sync.dma_start(out=LHS[r : r + 4, :], in_=LHS[0:4, :])
                nc.sync.dma_start(out=RHS[r : r + 4, :], in_=RHS[0:4, :])

        prep_ctx.close()
        main_ps = ctx.enter_context(
            tc.tile_pool(name="main_ps", bufs=8 if pack else 5, space="PSUM")
        )

        res = sb.tile([P, NT], f32)
        if nt_main < NT:
            nc.vector.memset(res, 0.0)

        rep_ctx = ExitStack()
        if reps > 1:  # timing amplification: re-execute the main loop
            rep_ctx.enter_context(tc.For_i(0, reps, 1))

        for t in range(nt_main):
            if pack:
                prev_init = None  # AP of the running min ([P,1]) or None
                for grp in range(NCH // 4):
                    pbs = []
                    for r in range(4):
                        j = 4 * grp + r
                        pb = main_ps.tile([P, CH], f32, tag="mm")
                        nc.tensor.matmul(
                            pb,
                            LHS[32 * r : 32 * r + 4, t * P : (t + 1) * P],
                            RHS[32 * r : 32 * r + 4, j * CH : (j + 1) * CH],
                            start=True,
                            stop=True,
                            tile_position=(32 * r, 0),
                        )
                        pbs.append(pb)
                    for h in range(2):
                        cp = scratch.tile([P, CH], f32, tag="cp")
                        nc.scalar.copy(cp, pbs[2 * h + 1])
                        dst = scratch.tile([P, CH], f32, tag="dst")
                        nc.vector.tensor_tensor_scan(
                            out=dst,
                            data0=pbs[2 * h],
                            initial=(1.0e30 if prev_init is None else prev_init),
                            data1=cp,
                            op0=mybir.AluOpType.min,
                            op1=mybir.AluOpType.min,
                        )
                        prev_init = dst[:, CH - 1 : CH]
                nc.vector.tensor_scalar(
                    out=res[:, t : t + 1],
                    in0=prev_init,
                    scalar1=xx[:, t : t + 1],
                    scalar2=1e-10,
                    op0=mybir.AluOpType.add,
                    op1=mybir.AluOpType.max,
                )
                continue
            if variant == "reduce2":
                # one free-axis min per chunk into mk16 columns; single merge
                mk16 = small.tile([P, NCH], f32, tag="mk16")
                for j in range(NCH):
                    pb = main_ps.tile([P, CH], f32, tag="mm")
                    nc.tensor.matmul(
                        pb,
                        LHS[:, t * P : (t + 1) * P],
                        RHS[:, j * CH : (j + 1) * CH],
                        start=True,
                        stop=True,
                    )
                    nc.vector.tensor_reduce(
                        out=mk16[:, j : j + 1], in_=pb,
                        axis=mybir.AxisListType.X, op=mybir.AluOpType.min,
                    )
                mrun2 = small.tile([P, 1], f32, tag="mrun2")
                nc.vector.tensor_reduce(
                    out=mrun2, in_=mk16,
                    axis=mybir.AxisListType.X, op=mybir.AluOpType.min,
                )
                nc.vector.tensor_scalar(
                    out=res[:, t : t + 1],
                    in0=mrun2,
                    scalar1=xx[:, t : t + 1],
                    scalar2=1e-10,
                    op0=mybir.AluOpType.add,
                    op1=mybir.AluOpType.max,
                )
                continue
            mrun = small.tile([P, 1], f32)
            mk8 = None
            if variant == "ttr_imm":
                mk8 = small.tile([P, PAIRS], f32, tag="mk8")
            for k in range(PAIRS):
                j0, j1 = 2 * k, 2 * k + 1
                pb0 = main_ps.tile([P, CH], f32, tag="mm")
                pb1 = main_ps.tile([P, CH], f32, tag="mm")
                nc.tensor.matmul(
                    pb0,
                    LHS[:, t * P : (t + 1) * P],
                    RHS[:, j0 * CH : (j0 + 1) * CH],
                    start=True,
                    stop=True,
                )
                nc.tensor.matmul(
                    pb1,
                    LHS[:, t * P : (t + 1) * P],
                    RHS[:, j1 * CH : (j1 + 1) * CH],
                    start=True,
                    stop=True,
                )
                if variant == "mmonly":
                    if k == 0:
                        nc.vector.memset(mrun, 7.0)
                    continue
                if variant == "reduce":
                    mk = scratch.tile([P, 2], f32, tag="mk")
                    nc.vector.tensor_reduce(
                        out=mk[:, 0:1], in_=pb0,
                        axis=mybir.AxisListType.X, op=mybir.AluOpType.min,
                    )
                    nc.vector.tensor_reduce(
                        out=mk[:, 1:2], in_=pb1,
                        axis=mybir.AxisListType.X, op=mybir.AluOpType.min,
                    )
                    if k == 0:
                        nc.vector.tensor_reduce(
                            out=mrun, in_=mk,
                            axis=mybir.AxisListType.X, op=mybir.AluOpType.min,
                        )
                    else:
                        mk2 = scratch.tile([P, 1], f32, tag="mk2")
                        nc.vector.tensor_reduce(
                            out=mk2, in_=mk,
                            axis=mybir.AxisListType.X, op=mybir.AluOpType.min,
                        )
                        nc.vector.tensor_tensor(
                            out=mrun, in0=mrun, in1=mk2, op=mybir.AluOpType.min
                        )
                    continue
                if variant == "actcopy":
                    cp = scratch.tile([P, CH], f32, tag="cp")
                    nc.scalar.copy(cp, pb1)
                    mk = scratch.tile([P, 2], f32, tag="mk")
                    nc.vector.tensor_reduce(
                        out=mk[:, 0:1], in_=pb0,
                        axis=mybir.AxisListType.X, op=mybir.AluOpType.min,
                    )
                    nc.vector.tensor_reduce(
                        out=mk[:, 1:2], in_=cp,
                        axis=mybir.AxisListType.X, op=mybir.AluOpType.min,
                    )
                    if k == 0:
                        nc.vector.tensor_reduce(
                            out=mrun, in_=mk,
                            axis=mybir.AxisListType.X, op=mybir.AluOpType.min,
                        )
                    else:
                        mk2 = scratch.tile([P, 1], f32, tag="mk2")
                        nc.vector.tensor_reduce(
                            out=mk2, in_=mk,
                            axis=mybir.AxisListType.X, op=mybir.AluOpType.min,
                        )
                        nc.vector.tensor_tensor(
                            out=mrun, in0=mrun, in1=mk2, op=mybir.AluOpType.min
                        )
                    continue
                cp = scratch.tile([P, CH], f32, tag="cp")
                nc.scalar.copy(cp, pb1)
                dst = scratch.tile([P, CH], f32, tag="dst")
                if variant == "ttr_imm":
                    nc.vector.tensor_tensor_reduce(
                        out=dst,
                        in0=pb0,
                        in1=cp,
                        scale=1.0,
                        scalar=1.0e30,
                        op0=mybir.AluOpType.min,
                        op1=mybir.AluOpType.min,
                        accum_out=mrun if k == 0 else mk8[:, k : k + 1],
                    )
                    continue
                nc.vector.tensor_tensor_reduce(
                    out=dst,
                    in0=pb0,
                    in1=cp,
                    scale=1.0,
                    scalar=(1.0e30 if k == 0 else mrun),
                    op0=mybir.AluOpType.min,
                    op1=mybir.AluOpType.min,
                    accum_out=mrun,
                )
            # res[:, t] = max(mrun + xx[:, t], 1e-10)
            nc.vector.tensor_scalar(
                out=res[:, t : t + 1],
                in0=mrun,
                scalar1=xx[:, t : t + 1],
                scalar2=1e-10,
                op0=mybir.AluOpType.add,
                op1=mybir.AluOpType.max,
            )

        rep_ctx.close()

        res1 = small.tile([P, 1], f32)
        nc.vector.tensor_reduce(
            out=res1, in_=res, axis=mybir.AxisListType.X, op=mybir.AluOpType.add
        )
        nc.sync.dma_start(out=out_dram.ap(), in_=res1)

    nc.compile()
    return nc


def _get_nc():
    key = "nc"
    if key not in _CACHE:
        _CACHE[key] = _build_nc()
    return _CACHE[key]


def kernel(ground_truth: np.ndarray, reconstruction: np.ndarray) -> np.ndarray:
    global LAST_RESULTS
    from concourse.bass_utils import run_bass_kernel_spmd

    gt = np.ascontiguousarray(ground_truth, dtype=np.float32)
    rc = np.ascontiguousarray(reconstruction, dtype=np.float32)
    B = gt.shape[0]
    assert gt.shape == (B, N, D) and rc.shape == (B, N, D)

    nc = _get_nc()

    in_maps = []
    for b in range(B):  # cores 0..3: min over reconstruction for each gt point
        in_maps.append({"a_pts": gt[b], "b_pts": rc[b]})
    for b in range(B):  # cores 4..7: min over gt for each reconstruction point
        in_maps.append({"a_pts": rc[b], "b_pts": gt[b]})

    try:
        results = run_bass_kernel_spmd(
            nc, in_maps, core_ids=list(range(2 * B)), trace=TRACE
        )
    except Exception:
        # transient NRT_EXEC_UNIT_UNRECOVERABLE has been observed after
        # heavy preceding runs; one retry recovers
        results = run_bass_kernel_spmd(
            nc, in_maps, core_ids=list(range(2 * B)), trace=TRACE
        )
    LAST_RESULTS = results

    partials = np.array(
        [float(np.sum(r["partial"].astype(np.float64))) for r in results.results]
    )
    loss_2 = partials[:B].sum() / (B * N)
    loss_1 = partials[B:].sum() / (B * N)
    total = (loss_1 + loss_2) * 1000.0
    return np.asarray(total, dtype=np.float32)



# revision 33
# speedup vs baseline: 3.1556x; 3.1556x over previous
"""Chamfer loss kernel for 8 Trainium2 NeuronCores.

Problem: ground_truth [4, 8192, 3], reconstruction [4, 8192, 3] (fp32).
  P[b,n,m] = ||x_n||^2 + ||y_m||^2 - 2 x_n.y_m
  loss = (mean(clamp(min_n P)) + mean(clamp(min_m P))) * 1000

Sharding: 8 independent (direction, batch) units -> 1 per core.
  cores 0..3: a = ground_truth[b],  b = reconstruction[b]   (loss_2: min over m)
  cores 4..7: a = reconstruction[b], b = ground_truth[b]    (loss_1: min over n)
Each core returns per-partition partial sums of clamp(min_b dist^2(a_i, b))
over its 8192 a-points; the host sums and combines.

Per-core kernel (variant "scan_pack3", mm dtype float32r):
  - min_b dist^2 = xx - 2 * max_b(x.y - yy/2): Q = x.y - yy/2 comes from a
    K=4 matmul with lhsT rows [x0,x1,x2,1] and rhs rows [y0,y1,y2,-yy/2] --
    raw coordinates on both sides, no operand scaling anywhere.
  - operands in float32r: bit-identical to fp32 but streams 1 column/cycle
    through the PE for free dim >= 256 vs 4 cycles/col for plain fp32 (the
    single biggest win: main loop 1086us -> 313us). Max |error| vs fp32
    matmul measured ~1e-4 absolute on P, fine for the 2e-2 mean tolerance.
  - 64 a-tiles x 16 b-chunks of [128, 512] matmuls into PSUM via
    tile_position row groups (operand rows replicated at partitions
    0/32/64/96).
  - max-reduce: ScalarE copies one PSUM bank of each pair to SBUF; VectorE
    tensor_tensor_scan(op0=max, op1=max) consumes (psum, sbuf) pairs at
    ~2 elements/lane/cycle; the scan's last column is the running max and
    feeds the next scan's `initial` directly. 512-wide ops only: scans
    whose PSUM operand crosses a 2KB bank boundary run ~2x slower
    (measured), and tensor_tensor_reduce crashes TRN2 silicon.
  - finalize per a-tile: res = xx - 2*max (DVE tensor_scalar); clamp at
    1e-10 batched once after the loop; final free-axis sum -> [128, 1]
    per-partition partial output, combined on host.
  - prep (~70us -> target lower): contiguous "(p t) d" staging DMAs (the
    induced a-point permutation is self-consistent between xx and LHS
    column order), batched per-coordinate PE transposes [128,64]->[64,128],
    Act evacuation, and 32 reshape+replicate DMAs alternating both HWDGE
    queues (SP + Activation). -yy/2 is folded into the squares op
    (scalar_tensor_tensor (b*-0.5)*b), the ones row into the staging
    pseudo-coordinate.
"""

import sys

if "/opt/trn_rl_repo" not in sys.path:
    sys.path.insert(0, "/opt/trn_rl_repo")

from contextlib import ExitStack

import numpy as np

N = 8192
D = 3
P = 128
NT = N // P  # 64 a-tiles
CH = 512
NCH = N // CH  # 16 b-chunks
PAIRS = NCH // 2

TRACE = False  # set True from test harness to capture an NTFF profile
LAST_RESULTS = None  # BassKernelResults of the most recent run (when traced)

_CACHE = {}


def _build_nc(
    mm_dtype_name="float32",
    nt_main=NT,
    skip_dma=False,
    variant="scan_pack",
    reps=1,
    scratch_bufs=3,
):
    import concourse.bacc as bacc
    import concourse.tile as tile
    from concourse import mybir
    from concourse.masks import make_identity

    f32 = mybir.dt.float32
    mm_dt = getattr(mybir.dt, mm_dtype_name)
    # float32r is bit-identical to float32 but selects the fast PE datapath
    # (1 col/cycle for free dim >= 256 vs 4 for plain fp32); operand tiles
    # are allocated as mm_dt and written by prep directly
    mm_view = lambda ap: ap

    nc = bacc.Bacc("TRN2", target_bir_lowering=False, debug=False)

    a_dram = nc.dram_tensor("a_pts", [N, D], f32, kind="ExternalInput")
    b_dram = nc.dram_tensor("b_pts", [N, D], f32, kind="ExternalInput")
    out_dram = nc.dram_tensor("partial", [P, 1], f32, kind="ExternalOutput")

    pack = variant in (
        "scan_pack",
        "scan_pack2",
        "scan_pack3",
        "scannocp",
        "packmm",
        "packmmcp",
        "packw",
    )
    sp3 = variant in ("scan_pack3", "preponly")
    # preponly / preponly0: For_i wraps the prep (prep timing probe) for the
    # sp3 / legacy prep styles respectively
    rep_all = variant in ("preponly", "preponly0")
    if rep_all:
        pack = True
        nt_main = 0
    with tile.TileContext(nc) as tc, ExitStack() as ctx:
        consts = ctx.enter_context(tc.tile_pool(name="consts", bufs=1))
        sb = ctx.enter_context(tc.tile_pool(name="sb", bufs=1))
        small = ctx.enter_context(tc.tile_pool(name="small", bufs=2))
        scratch = ctx.enter_context(tc.tile_pool(name="scratch", bufs=scratch_bufs))
        sq3p = ctx.enter_context(tc.tile_pool(name="sq3p", bufs=2))
        prep_ctx = ExitStack()
        prep_ps = prep_ctx.enter_context(tc.tile_pool(name="prep_ps", bufs=2, space="PSUM"))
        yy_ps = prep_ctx.enter_context(tc.tile_pool(name="yy_ps", bufs=1, space="PSUM"))
        main_ps = None  # opened after prep pools close when pack=True

        ident = consts.tile([P, P], f32)
        make_identity(nc, ident)
        if not sp3:
            # ones_mask [3, 4]: col 3 = 1s, cols 0:2 = 0 -> ones_mask.T @ sq3
            # puts yy into row 3 and 0 into rows 0:2
            ones_mask = consts.tile([3, 4], f32)
            nc.vector.memset(ones_mask, 0.0)
            nc.vector.memset(ones_mask[:, 3:4], 1.0)
            # per-partition scale [-2, -2, -2, 1] for the RHS write
            sv = consts.tile([4, 1], f32)
            nc.vector.memset(sv, -2.0)
            nc.gpsimd.affine_select(
                out=sv,
                in_=sv,
                compare_op=mybir.AluOpType.not_equal,
                fill=1.0,
                base=-3,
                pattern=[[0, 1]],
                channel_multiplier=1,
            )

        rep_ctx = ExitStack()
        if rep_all and reps > 1:
            rep_ctx.enter_context(tc.For_i(0, reps, 1))

        # staging: [128, 64 tiles-per-partition, 4]. For sp3 the DRAM side is
        # "(p t) d" so each partition reads one contiguous 768B run; the
        # resulting a-point permutation is consistent between xx and the
        # LHS column order (both derive from the same staging), and b-point
        # order is free (min over all b). 4th pseudo-coord: a-side = 1.0,
        # b-side = yy (sp3: -yy/2 computed below; legacy: 0).
        astage = sb.tile([P, NT, 4], f32)
        nc.vector.memset(astage[:, :, 3:4], 1.0)
        bstage = sb.tile([P, NT, 4], f32)
        if not sp3:
            nc.vector.memset(bstage[:, :, 3:4], 0.0)
        if skip_dma:
            nc.vector.memset(astage[:, :, 0:D], 0.5)
            nc.vector.memset(bstage[:, :, 0:D], 0.25)
        else:
            pattern = "(p t) d -> p t d" if sp3 else "(t p) d -> p t d"
            nc.sync.dma_start(
                out=astage[:, :, 0:D],
                in_=a_dram.ap().rearrange(pattern, p=P),
            )
            # b side on the second HWDGE queue (Activation) to overlap
            bq = nc.scalar if sp3 else nc.sync
            bq.dma_start(
                out=bstage[:, :, 0:D],
                in_=b_dram.ap().rearrange(pattern, p=P),
            )

        # xx per a-point, natural layout [128, 64]
        sqa = sb.tile([P, NT, D], f32)
        nc.vector.tensor_mul(sqa, astage[:, :, 0:D], astage[:, :, 0:D])
        xx = sb.tile([P, NT], f32)
        nc.vector.tensor_reduce(
            out=xx, in_=sqa, axis=mybir.AxisListType.X, op=mybir.AluOpType.add
        )

        # K-major operand buffers (full 128 partitions when packing: rows
        # replicated at partition bases 0/32/64/96)
        LHS = sb.tile([P if pack else 4, N], mm_dt)  # rows: x0, x1, x2, 1
        RHS = sb.tile([P if pack else 4, N], mm_dt)  # rows: -2y0, -2y1, -2y2, yy

        if sp3:
            # -yy/2 into bstage[:, :, 3]: (-b/2 * b) summed over coords
            sqb = sb.tile([P, NT, D], f32)
            nc.vector.scalar_tensor_tensor(
                out=sqb,
                in0=bstage[:, :, 0:D],
                scalar=-0.5,
                in1=bstage[:, :, 0:D],
                op0=mybir.AluOpType.mult,
                op1=mybir.AluOpType.mult,
            )
            nc.vector.tensor_reduce(
                out=bstage[:, :, 3],
                in_=sqb,
                axis=mybir.AxisListType.X,
                op=mybir.AluOpType.add,
            )
            # per pseudo-coordinate: transpose [128, 64] -> [64, 128] (PE),
            # evacuate to SBUF (Act), reshape-DMA into operand row d at all 4
            # tile_position row groups directly, alternating both HWDGE
            # queues so the 32 small DMAs pipeline
            qs = (nc.sync, nc.scalar)
            qi = 0
            for d in range(4):
                for stage, DST in ((astage, LHS), (bstage, RHS)):
                    tp3 = prep_ps.tile([NT, P], f32, tag="tp3")
                    nc.tensor.transpose(tp3, stage[:, :, d], ident)
                    st = sq3p.tile([NT, P], mm_dt, tag="st", bufs=4)
                    nc.scalar.copy(st, tp3)
                    for r in (0, 32, 64, 96):
                        qs[qi % 2].dma_start(
                            out=DST[d + r : d + r + 1, :], in_=st
                        )
                        qi += 1

        # build LHS (a-coords + ones row) and RHS (b-side) per 512-col group
        for g in range(NCH if not sp3 else 0):
            # a-side: transpose 4 [128,4] tiles -> psum [4, 512], copy to LHS
            tpa = prep_ps.tile([4, CH], f32, tag="tp")
            for c in range(4):
                t = 4 * g + c
                nc.tensor.transpose(tpa[:, c * P : (c + 1) * P], astage[:, t, :], ident)
            nc.scalar.copy(LHS[0:4, g * CH : (g + 1) * CH], tpa)

            # b-side: transpose -> [y0,y1,y2,0]; square rows 0:3; accumulate
            # yy into row 3 via ones_mask matmul; write RHS with scale vec
            tpb = prep_ps.tile([4, CH], f32, tag="tp")
            for c in range(4):
                t = 4 * g + c
                nc.tensor.transpose(tpb[:, c * P : (c + 1) * P], bstage[:, t, :], ident)
            sq3 = sq3p.tile([3, CH], f32)
            nc.scalar.square(sq3, tpb[0:3, :])
            ypb = yy_ps.tile([4, CH], f32)
            nc.tensor.matmul(ypb, ones_mask, sq3, start=True, stop=True)
            ypb_sb = sq3p.tile([4, CH], f32, tag="ypb_sb")
            nc.scalar.copy(ypb_sb, ypb)
            # RHS = tpb * [-2,-2,-2,1] + [0,0,0,yy]  (<=1 PSUM operand per op)
            nc.vector.scalar_tensor_tensor(
                out=RHS[0:4, g * CH : (g + 1) * CH],
                in0=tpb,
                scalar=sv,
                in1=ypb_sb,
                op0=mybir.AluOpType.mult,
                op1=mybir.AluOpType.add,
            )

        if pack and not sp3:
            # replicate operand rows to partition bases 32/64/96 for
            # tile_position row-group packing (4 concurrent K=4 matmuls);
            # sp3 replicates directly from the reshape staging above
            for r in (32, 64, 96):
                nc.sync.dma_start(out=LHS[r : r + 4, :], in_=LHS[0:4, :])
                nc.sync.dma_start(out=RHS[r : r + 4, :], in_=RHS[0:4, :])

        if rep_all:
            rep_ctx.close()
        prep_ctx.close()
        main_ps = ctx.enter_context(
            tc.tile_pool(
                name="main_ps",
                bufs=2 if variant == "packw" else (8 if pack else 5),
                space="PSUM",
            )
        )

        res = sb.tile([P, NT], f32)
        if nt_main < NT or variant in ("packmm", "packmmcp"):
            nc.vector.memset(res, 0.0)
        garb = None
        if variant == "scannocp":
            garb = sb.tile([P, CH], f32)
            nc.vector.memset(garb, 1.0e30)

        if not rep_all and reps > 1:  # timing amplification: re-execute main loop
            rep_ctx.enter_context(tc.For_i(0, reps, 1))

        for t in range(nt_main):
            if variant == "packw":
                # 1024-wide scans: 2 PSUM banks per operand tile; each quad
                # covers 4 b-chunks (2 direct from PSUM + 2 via Act copy)
                prev_init = None
                for q in range(4):
                    pbA = main_ps.tile([P, 2 * CH], f32, tag="mmA")
                    pbB = main_ps.tile([P, 2 * CH], f32, tag="mmB")
                    for half, pb in ((0, pbA), (1, pbB)):
                        for c in range(2):
                            j = 4 * q + 2 * half + c
                            r = j % 4
                            nc.tensor.matmul(
                                pb[:, c * CH : (c + 1) * CH],
                                mm_view(LHS[32 * r : 32 * r + 4, t * P : (t + 1) * P]),
                                mm_view(RHS[32 * r : 32 * r + 4, j * CH : (j + 1) * CH]),
                                start=True,
                                stop=True,
                                tile_position=(32 * r, 0),
                            )
                    cp = scratch.tile([P, 2 * CH], f32, tag="cp")
                    nc.scalar.copy(cp, pbB)
                    dst = scratch.tile([P, 2 * CH], f32, tag="dst")
                    nc.vector.tensor_tensor_scan(
                        out=dst,
                        data0=pbA,
                        initial=(1.0e30 if prev_init is None else prev_init),
                        data1=cp,
                        op0=mybir.AluOpType.min,
                        op1=mybir.AluOpType.min,
                    )
                    prev_init = dst[:, 2 * CH - 1 : 2 * CH]
                nc.vector.tensor_scalar(
                    out=res[:, t : t + 1],
                    in0=prev_init,
                    scalar1=xx[:, t : t + 1],
                    scalar2=1e-10,
                    op0=mybir.AluOpType.add,
                    op1=mybir.AluOpType.max,
                )
                continue
            if pack:
                prev_init = None  # AP of the running min ([P,1]) or None
                for grp in range(NCH // 4):
                    pbs = []
                    for r in range(4):
                        j = 4 * grp + r
                        pb = main_ps.tile([P, CH], f32, tag="mm")
                        nc.tensor.matmul(
                            pb,
                            mm_view(LHS[32 * r : 32 * r + 4, t * P : (t + 1) * P]),
                            mm_view(RHS[32 * r : 32 * r + 4, j * CH : (j + 1) * CH]),
                            start=True,
                            stop=True,
                            tile_position=(32 * r, 0),
                        )
                        pbs.append(pb)
                    if variant == "packmm":
                        continue
                    if variant == "scannocp":
                        # diagnostic: every PSUM bank scanned directly, no Act
                        # copies (data1 = constant SBUF garbage)
                        for h in range(4):
                            dst = scratch.tile([P, CH], f32, tag="dst")
                            nc.vector.tensor_tensor_scan(
                                out=dst,
                                data0=pbs[h],
                                initial=(1.0e30 if prev_init is None else prev_init),
                                data1=garb,
                                op0=mybir.AluOpType.min,
                                op1=mybir.AluOpType.min,
                            )
                            prev_init = dst[:, CH - 1 : CH]
                        continue
                    for h in range(2):
                        cp = scratch.tile([P, CH], f32, tag="cp")
                        nc.scalar.copy(cp, pbs[2 * h + 1])
                        if variant == "packmmcp":
                            continue
                        dst = scratch.tile([P, CH], f32, tag="dst")
                        scan_op = (
                            mybir.AluOpType.max if sp3 else mybir.AluOpType.min
                        )
                        nc.vector.tensor_tensor_scan(
                            out=dst,
                            data0=pbs[2 * h],
                            initial=(
                                ((-1.0e30) if sp3 else 1.0e30)
                                if prev_init is None
                                else prev_init
                            ),
                            data1=cp,
                            op0=scan_op,
                            op1=scan_op,
                        )
                        prev_init = dst[:, CH - 1 : CH]
                if variant in ("packmm", "packmmcp"):
                    continue
                if sp3:
                    # res[:, t] = xx - 2 * max_m(x.y - yy/2)  (= min distance^2;
                    # clamp batched after the loop)
                    nc.vector.tensor_scalar(
                        out=res[:, t : t + 1],
                        in0=prev_init,
                        scalar1=-2.0,
                        scalar2=xx[:, t : t + 1],
                        op0=mybir.AluOpType.mult,
                        op1=mybir.AluOpType.add,
                    )
                    continue
                if variant in ("scan_pack2", "scannocp"):
                    # per-tile combine on Act (Identity: in*1 + bias) keeps the
                    # DVE stream pure scans; clamp happens batched at the end
                    nc.scalar.activation(
                        out=res[:, t : t + 1],
                        in_=prev_init,
                        func=mybir.ActivationFunctionType.Identity,
                        bias=xx[:, t : t + 1],
                        scale=1.0,
                    )
                else:
                    nc.vector.tensor_scalar(
                        out=res[:, t : t + 1],
                        in0=prev_init,
                        scalar1=xx[:, t : t + 1],
                        scalar2=1e-10,
                        op0=mybir.AluOpType.add,
                        op1=mybir.AluOpType.max,
                    )
                continue
            if variant == "reduce2":
                # one free-axis min per chunk into mk16 columns; single merge
                mk16 = small.tile([P, NCH], f32, tag="mk16")
                for j in range(NCH):
                    pb = main_ps.tile([P, CH], f32, tag="mm")
                    nc.tensor.matmul(
                        pb,
                        mm_view(LHS[:, t * P : (t + 1) * P]),
                        mm_view(RHS[:, j * CH : (j + 1) * CH]),
                        start=True,
                        stop=True,
                    )
                    nc.vector.tensor_reduce(
                        out=mk16[:, j : j + 1], in_=pb,
                        axis=mybir.AxisListType.X, op=mybir.AluOpType.min,
                    )
                mrun2 = small.tile([P, 1], f32, tag="mrun2")
                nc.vector.tensor_reduce(
                    out=mrun2, in_=mk16,
                    axis=mybir.AxisListType.X, op=mybir.AluOpType.min,
                )
                nc.vector.tensor_scalar(
                    out=res[:, t : t + 1],
                    in0=mrun2,
                    scalar1=xx[:, t : t + 1],
                    scalar2=1e-10,
                    op0=mybir.AluOpType.add,
                    op1=mybir.AluOpType.max,
                )
                continue
            mrun = small.tile([P, 1], f32)
            mk8 = None
            if variant == "ttr_imm":
                mk8 = small.tile([P, PAIRS], f32, tag="mk8")
            for k in range(PAIRS):
                j0, j1 = 2 * k, 2 * k + 1
                pb0 = main_ps.tile([P, CH], f32, tag="mm")
                pb1 = main_ps.tile([P, CH], f32, tag="mm")
                nc.tensor.matmul(
                    pb0,
                    mm_view(LHS[:, t * P : (t + 1) * P]),
                    mm_view(RHS[:, j0 * CH : (j0 + 1) * CH]),
                    start=True,
                    stop=True,
                )
                nc.tensor.matmul(
                    pb1,
                    mm_view(LHS[:, t * P : (t + 1) * P]),
                    mm_view(RHS[:, j1 * CH : (j1 + 1) * CH]),
                    start=True,
                    stop=True,
                )
                if variant == "mmonly":
                    if k == 0:
                        nc.vector.memset(mrun, 7.0)
                    continue
                if variant == "reduce":
                    mk = scratch.tile([P, 2], f32, tag="mk")
                    nc.vector.tensor_reduce(
                        out=mk[:, 0:1], in_=pb0,
                        axis=mybir.AxisListType.X, op=mybir.AluOpType.min,
                    )
                    nc.vector.tensor_reduce(
                        out=mk[:, 1:2], in_=pb1,
                        axis=mybir.AxisListType.X, op=mybir.AluOpType.min,
                    )
                    if k == 0:
                        nc.vector.tensor_reduce(
                            out=mrun, in_=mk,
                            axis=mybir.AxisListType.X, op=mybir.AluOpType.min,
                        )
                    else:
                        mk2 = scratch.tile([P, 1], f32, tag="mk2")
                        nc.vector.tensor_reduce(
                            out=mk2, in_=mk,
                            axis=mybir.AxisListType.X, op=mybir.AluOpType.min,
                        )
                        nc.vector.tensor_tensor(
                            out=mrun, in0=mrun, in1=mk2, op=mybir.AluOpType.min
                        )
                    continue
                if variant == "actcopy":
                    cp = scratch.tile([P, CH], f32, tag="cp")
                    nc.scalar.copy(cp, pb1)
                    mk = scratch.tile([P, 2], f32, tag="mk")
                    nc.vector.tensor_reduce(
                        out=mk[:, 0:1], in_=pb0,
                        axis=mybir.AxisListType.X, op=mybir.AluOpType.min,
                    )
                    nc.vector.tensor_reduce(
                        out=mk[:, 1:2], in_=cp,
                        axis=mybir.AxisListType.X, op=mybir.AluOpType.min,
                    )
                    if k == 0:
                        nc.vector.tensor_reduce(
                            out=mrun, in_=mk,
                            axis=mybir.AxisListType.X, op=mybir.AluOpType.min,
                        )
                    else:
                        mk2 = scratch.tile([P, 1], f32, tag="mk2")
                        nc.vector.tensor_reduce(
                            out=mk2, in_=mk,
                            axis=mybir.AxisListType.X, op=mybir.AluOpType.min,
                        )
                        nc.vector.tensor_tensor(
                            out=mrun, in0=mrun, in1=mk2, op=mybir.AluOpType.min
                        )
                    continue
                cp = scratch.tile([P, CH], f32, tag="cp")
                nc.scalar.copy(cp, pb1)
                dst = scratch.tile([P, CH], f32, tag="dst")
                if variant == "ttr_imm":
                    nc.vector.tensor_tensor_reduce(
                        out=dst,
                        in0=pb0,
                        in1=cp,
                        scale=1.0,
                        scalar=1.0e30,
                        op0=mybir.AluOpType.min,
                        op1=mybir.AluOpType.min,
                        accum_out=mrun if k == 0 else mk8[:, k : k + 1],
                    )
                    continue
                nc.vector.tensor_tensor_reduce(
                    out=dst,
                    in0=pb0,
                    in1=cp,
                    scale=1.0,
                    scalar=(1.0e30 if k == 0 else mrun),
                    op0=mybir.AluOpType.min,
                    op1=mybir.AluOpType.min,
                    accum_out=mrun,
                )
            # res[:, t] = max(mrun + xx[:, t], 1e-10)
            nc.vector.tensor_scalar(
                out=res[:, t : t + 1],
                in0=mrun,
                scalar1=xx[:, t : t + 1],
                scalar2=1e-10,
                op0=mybir.AluOpType.add,
                op1=mybir.AluOpType.max,
            )

        rep_ctx.close()

        if variant in ("scan_pack2", "scan_pack3", "scannocp"):
            resc = small.tile([P, NT], f32, tag="resc")
            nc.vector.tensor_scalar(
                out=resc,
                in0=res,
                scalar1=1e-10,
                scalar2=None,
                op0=mybir.AluOpType.max,
            )
            res = resc
        res1 = small.tile([P, 1], f32)
        nc.vector.tensor_reduce(
            out=res1, in_=res, axis=mybir.AxisListType.X, op=mybir.AluOpType.add
        )
        nc.sync.dma_start(out=out_dram.ap(), in_=res1)

    nc.compile()
    return nc


def _get_nc():
    key = "nc"
    if key not in _CACHE:
        _CACHE[key] = _build_nc(mm_dtype_name="float32r", variant="scan_pack3")
    return _CACHE[key]


def kernel(ground_truth: np.ndarray, reconstruction: np.ndarray) -> np.ndarray:
    global LAST_RESULTS
    from concourse.bass_utils import run_bass_kernel_spmd

    gt = np.ascontiguousarray(ground_truth, dtype=np.float32)
    rc = np.ascontiguousarray(reconstruction, dtype=np.float32)
    B = gt.shape[0]
    assert gt.shape == (B, N, D) and rc.shape == (B, N, D)

    nc = _get_nc()

    in_maps = []
    for b in range(B):  # cores 0..3: min over reconstruction for each gt point
        in_maps.append({"a_pts": gt[b], "b_pts": rc[b]})
    for b in range(B):  # cores 4..7: min over gt for each reconstruction point
        in_maps.append({"a_pts": rc[b], "b_pts": gt[b]})

    try:
        results = run_bass_kernel_spmd(
            nc, in_maps, core_ids=list(range(2 * B)), trace=TRACE
        )
    except Exception:
        # transient NRT_EXEC_UNIT_UNRECOVERABLE has been observed after
        # heavy preceding runs; one retry recovers
        results = run_bass_kernel_spmd(
            nc, in_maps, core_ids=list(range(2 * B)), trace=TRACE
        )
    LAST_RESULTS = results

    partials = np.array(
        [float(np.sum(r["partial"].astype(np.float64))) for r in results.results]
    )
    loss_2 = partials[:B].sum() / (B * N)
    loss_1 = partials[B:].sum() / (B * N)
    total = (loss_1 + loss_2) * 1000.0
    return np.asarray(total, dtype=np.float32)

